# revision 17
# baseline (speedup 1.0000x reference)
"""Trainium2 Bass kernel for LogSpaceMinGRU.

Math: the reference computes, per (batch, channel), a log-space Heinsen scan:
    hg = x @ W.T ; hidden, gate = split(hg)
    log_h = cumulative-logsumexp formulation of  h_t = (1-z_t) h_{t-1} + z_t g(hidden_t)
    out = exp(log_h)
with z = sigmoid(gate), g(x) = relu(x)+0.5 (x>=0) | sigmoid(x) (x<0).

The log-space form exists only for numerical stability.  In linear space the
recurrence h_t = c_t*h_{t-1} + v_t (c = sigmoid(-gate) in (0,1),
v = z*g >= 0) is a convex-combination update, perfectly stable in f32, and
maps 1:1 onto the TRN2 DVE `tensor_tensor_scan` instruction
(state = data0*state + data1 along the free dim).  Verified numerically:
the linear-space f32 result is ~7e-7 from the f64 ground truth, while the
f32 log-space reference itself carries ~2e-4 of rounding error.

Note g(x) = max(sigmoid(x), x + 0.5) for all x (equality at 0; sigmoid is
above the line for x<0, below for x>=0), which gives a branch-free form.

Sharding over 8 cores: batch (4) x output-feature-half (2).  Each core
computes, for one batch b and one 512-wide feature slice:
    hg_slice = x[b] @ W_slice.T  -> [4096, 1024] (512 hidden | 512 gate)
    h = scan(...)                -> [512, 4096] (channel-major)
The host pre-transposes x[b] to [d, s] (free) and post-transposes the
channel-major output back, so the device never pays for transposes.

Optimization notes (second session, measured with the For_i loop-count
differential in kbench.py -- per-call axon/PJRT dispatch noise is
seconds, so in-NEFF hardware loops are the only reliable clock):

  * The kernel is TENSOR-bound and sits AT the PE serial floor.  On this
    TRN2, matmul time fits t_mm ~= (K_loadrows + N_out_rows) * 0.417ns:
    LoadStationary does NOT overlap compute (measured: f16 [K128,N512]
    262-282ns vs 267 model; fp8-DR [K256,N512] 211ns vs 213 model).
    Per core: 109us compute + 27us weight loads = 136.6us model vs
    134.1us measured for this kernel.  There is no weight-reuse path:
    same-lhsT streams don't elide loads, and flipping
    InstMatmult.ldweights / explicit nc.tensor.ldweights changes
    nothing (ldwprobe.py; walrus runs with --enable-ldw-opt=false).
  * fp8e4 + MatmulPerfMode.DoubleRow works (numerics verified on HW,
    _build_program_fp8 below) and halves compute rows, but accurate
    output needs a 3-pass residual split (x1@W1+x2@W1+x1@W2, host
    relerr 1.37e-2 vs the 2e-2 gate; 2-pass is 2.17e-2 and fails; see
    fp8_acc*.py).  Measured on the full kernel the 3-pass variant runs
    183-204us -- worse even than the 164us serial model, and far worse
    than f16's 119-134us.  fp8 cannot beat f16 at passing accuracy.
  * Loop-barrier extrapolation (1 vs 2 passes per For_i iteration:
    134.1 -> 126.4us) puts the true steady state at ~119us, which
    corroborates the 120us differential estimate from session 1.  The
    f16 kernel hides most of the weight-load tax in context (119 vs
    109us compute-only floor): ~8% of genuine headroom remains, but no
    instruction-level mechanism on this toolchain reaches it.
  * Interleaving the 4 psum accumulation chains per p (minter=True)
    helps isolated matmul streams ~10% but regresses the real kernel to
    204us -- keep the sequential chain order.
"""

import sys

sys.path.insert(0, "/opt/trn_rl_repo")

import numpy as np

_B, _S, _D = 4, 4096, 1024
_CH = 512          # channels per core (feature slice)
_Q = 1024          # sequence chunk ("quarter" of S)
_NQ = _S // _Q     # 4
_NK = _D // 128    # 8 contraction tiles
_NP = _CH // 128   # 4 channel tiles (pairs of hidden/gate e-chunks)

_programs = {}


def _build_program(reps=1, mm16=False, mm_only=False, xbufs=2, sbufs=2, hbufs=2,
                   Q=1024, psbufs=2, zpool=False, kin=False, xres=False,
                   xmerge=False, psplit=False, loop=0, minter=False):
    import contextlib
    import concourse.bass as bass  # noqa: F401  (registers engine classes)
    import concourse.tile as tile
    from concourse import bacc, mybir

    f32 = mybir.dt.float32
    f32r = mybir.dt.float16 if mm16 else mybir.dt.float32r
    AF = mybir.ActivationFunctionType
    OP = mybir.AluOpType
    _Q = Q               # shadow module defaults for this build
    _NQ = _S // Q

    nc = bacc.Bacc("TRN2", target_bir_lowering=False, debug=False)
    x_d = nc.dram_tensor("x", [_D, _S], f32r, kind="ExternalInput").ap()
    w_d = nc.dram_tensor("w", [_D, 2 * _CH], f32r, kind="ExternalInput").ap()
    h_d = nc.dram_tensor("h", [_CH, _S], f32, kind="ExternalOutput").ap()

    with tile.TileContext(nc) as tc:
        with (
            tc.tile_pool(name="wp", bufs=1) as wp,
            tc.tile_pool(name="xp", bufs=xbufs) as xp,
            tc.tile_pool(name="ps", bufs=psbufs, space="PSUM") as ps,
            tc.tile_pool(name="sb", bufs=sbufs) as sb,
            tc.tile_pool(name="hp", bufs=hbufs) as hp,
        ):
            # resident weights: [d, e_local] as 8 k-tiles of [128, 1024]
            wt = []
            for k in range(_NK):
                t = wp.tile([128, 2 * _CH], f32r, tag=f"w{k}")
                nc.sync.dma_start(t[:], w_d[k * 128 : (k + 1) * 128, :])
                wt.append(t)

            xr = []
            if xres:
                # whole x resident: 8 DMAs of 1 MiB at full bandwidth
                for k in range(_NK):
                    t = wp.tile([128, _S], f32r, tag=f"xr{k}")
                    nc.sync.dma_start(t[:], x_d[k * 128 : (k + 1) * 128, :])
                    xr.append(t)

            hprev = [None] * _NP

            def _body_quarter(q):
                sq = slice(q * _Q, (q + 1) * _Q)
                if xres:
                    xq = xr
                    xoff = q * _Q
                elif xmerge:
                    # one 3D-AP DMA for the whole quarter: [p, k, j] from
                    # xT viewed as [(k p), s]; tile holds k-chunks side by side
                    t = xp.tile([128, _NK * _Q], f32r, tag="xm")
                    src = x_d.rearrange("(k p) s -> p k s", p=128)
                    nc.sync.dma_start(
                        t[:].rearrange("p (k j) -> p k j", k=_NK),
                        src[:, :, sq],
                    )
                    xq = [t[:, k * _Q : (k + 1) * _Q] for k in range(_NK)]
                    xoff = 0
                else:
                    xq = []
                    xoff = 0
                    for k in range(_NK):
                        t = xp.tile([128, _Q], f32r, tag=f"x{k}")
                        nc.sync.dma_start(t[:], x_d[k * 128 : (k + 1) * 128, sq])
                        xq.append(t)
                for p in range(_NP):
                    if psplit:
                        psh = [ps.tile([128, 512], f32, tag=f"ph{h}",
                                       name=f"psh{h}")
                               for h in range(_Q // 512)]
                        psg = [ps.tile([128, 512], f32, tag=f"pg{h}",
                                       name=f"psg{h}")
                               for h in range(_Q // 512)]
                    else:
                        psh = ps.tile([128, _Q], f32, tag="ph")
                        psg = ps.tile([128, _Q], f32, tag="pg")
                    if minter:
                        # round-robin the 4 independent accumulation chains
                        # (2 e-chunks x 2 halves) so consecutive PE
                        # instructions never depend on each other: hides
                        # psum RAW latency (measured ~10% on mm probes).
                        assert psplit
                        chains = []
                        for ec, pst in ((p, psh), (_NP + p, psg)):
                            wcol = slice(ec * 128, (ec + 1) * 128)
                            for half in range(_Q // 512):
                                xs2 = slice(xoff + half * 512,
                                            xoff + (half + 1) * 512)
                                chains.append((pst[half], wcol, xs2))
                        for k in range(_NK):
                            for psth, wcol, xs2 in chains:
                                nc.tensor.matmul(
                                    psth[:],
                                    wt[k][:, wcol],
                                    xq[k][:, xs2],
                                    start=(k == 0),
                                    stop=(k == _NK - 1),
                                    skip_group_check=True,
                                )
                    for ec, pst in ((p, psh), (_NP + p, psg)):
                        if minter:
                            break
                        wcol = slice(ec * 128, (ec + 1) * 128)
                        if kin:
                            # k-outer: consecutive matmuls share lhsT, giving
                            # codegen/hardware a shot at eliding weight reloads
                            for k in range(_NK):
                                for half in range(_Q // 512):
                                    sh2 = slice(half * 512, (half + 1) * 512)
                                    xs2 = slice(xoff + half * 512,
                                                xoff + (half + 1) * 512)
                                    out_ap = pst[half][:] if psplit else pst[:, sh2]
                                    nc.tensor.matmul(
                                        out_ap,
                                        wt[k][:, wcol],
                                        xq[k][:, xs2],
                                        start=(k == 0),
                                        stop=(k == _NK - 1),
                                        skip_group_check=True,
                                    )
                        else:
                            for half in range(_Q // 512):
                                sh2 = slice(half * 512, (half + 1) * 512)
                                xs2 = slice(xoff + half * 512,
                                            xoff + (half + 1) * 512)
                                out_ap = pst[half] if psplit else pst[:, sh2]
                                for k in range(_NK):
                                    nc.tensor.matmul(
                                        out_ap[:] if psplit else out_ap,
                                        wt[k][:, wcol],
                                        xq[k][:, xs2],
                                        start=(k == 0),
                                        stop=(k == _NK - 1),
                                    )
                    if psplit:
                        # per-bank consumers: ACT/DVE start on bank 0 while
                        # PE still fills bank 1
                        sh = sb.tile([128, _Q], f32, tag="sh")
                        g = sb.tile([128, _Q], f32, tag="g")
                        cc = sb.tile([128, _Q], f32, tag="c")
                        z = sb.tile([128, _Q], f32, tag="z")
                        v = sb.tile([128, _Q], f32, tag="v")
                        for hh in range(_Q // 512):
                            hs = slice(hh * 512, (hh + 1) * 512)
                            nc.scalar.activation(sh[:, hs], psh[hh][:], AF.Sigmoid)
                            nc.vector.scalar_tensor_tensor(
                                g[:, hs], psh[hh][:], 0.5, sh[:, hs],
                                OP.add, OP.max
                            )
                            nc.scalar.activation(
                                cc[:, hs], psg[hh][:], AF.Sigmoid, scale=-1.0
                            )
                            nc.gpsimd.tensor_scalar(
                                z[:, hs], cc[:, hs], -1.0, 1.0, OP.mult, OP.add
                            )
                            nc.gpsimd.tensor_mul(v[:, hs], z[:, hs], g[:, hs])
                        h = hp.tile([128, _Q], f32, tag=f"h{p}")
                        init = 0.0 if q == 0 else hprev[p][:, _Q - 1 : _Q]
                        nc.vector.tensor_tensor_scan(
                            h[:], cc[:], v[:], init, OP.mult, OP.add
                        )
                        hprev[p] = h
                        nc.sync.dma_start(h_d[p * 128 : (p + 1) * 128, sq], h[:])
                        continue
                    sh = sb.tile([128, _Q], f32, tag="sh")
                    nc.scalar.activation(sh[:], psh[:], AF.Sigmoid)
                    if mm_only:
                        # bench variant: consume psum banks cheaply, skip the
                        # rest of the pipeline
                        sg2 = sb.tile([128, _Q], f32, tag="sg2")
                        nc.scalar.activation(sg2[:], psg[:], AF.Sigmoid)
                        nc.sync.dma_start(h_d[p * 128 : (p + 1) * 128, sq], sh[:])
                        continue
                    g = sb.tile([128, _Q], f32, tag="g")
                    nc.vector.scalar_tensor_tensor(
                        g[:], psh[:], 0.5, sh[:], OP.add, OP.max
                    )
                    cc = sb.tile([128, _Q], f32, tag="c")
                    nc.scalar.activation(cc[:], psg[:], AF.Sigmoid, scale=-1.0)
                    z = sb.tile([128, _Q], f32, tag="z")
                    if zpool:
                        # z = 1 - c on Pool: one tensor_scalar, saves an ACT pass
                        nc.gpsimd.tensor_scalar(
                            z[:], cc[:], -1.0, 1.0, OP.mult, OP.add
                        )
                    else:
                        nc.scalar.activation(z[:], psg[:], AF.Sigmoid)
                    v = sb.tile([128, _Q], f32, tag="v")
                    nc.gpsimd.tensor_mul(v[:], z[:], g[:])
                    h = hp.tile([128, _Q], f32, tag=f"h{p}")
                    init = 0.0 if q == 0 else hprev[p][:, _Q - 1 : _Q]
                    nc.vector.tensor_tensor_scan(
                        h[:], cc[:], v[:], init, OP.mult, OP.add
                    )
                    hprev[p] = h
                    nc.sync.dma_start(h_d[p * 128 : (p + 1) * 128, sq], h[:])

            loop_cm = tc.For_i(0, loop) if loop else contextlib.nullcontext()
            with loop_cm:
                for q in range(_NQ * reps):
                    _body_quarter(q % _NQ)

    nc.compile()
    return nc


def _build_program_fp8(reps=1, passes=3, xbufs=2, sbufs=2, hbufs=2, psbufs=2,
                       korder=False, loop=0):
    """fp8(e4m3) DoubleRow variant: hg = x1@W1 + x2@W1 + x1@W2 where
    x = x1 + x2 and W = W1 + W2 are host-side fp8 digit splits (3-pass
    residual correction, host-measured relerr 1.37e-2 vs the 2e-2 gate).
    Each DoubleRow matmul contracts 2x128 K at 0.5 cycles/row.

    Host layouts (per core):
      x1,x2: [512 (kp p), 4*2048 (q two j)]  fp8  -- quarter-paired
      w1,w2: [512 (kp p), 2048 (two e)]      fp8
      h:     [512, 4096] f32
    """
    import contextlib
    import concourse.bass as bass  # noqa: F401
    import concourse.tile as tile
    from concourse import bacc, mybir

    f32 = mybir.dt.float32
    f8 = mybir.dt.float8e4
    AF = mybir.ActivationFunctionType
    OP = mybir.AluOpType
    PM = mybir.MatmulPerfMode.DoubleRow
    Q = 1024
    NQ = _S // Q
    NKP = _D // 256  # 4 pair tiles

    nc = bacc.Bacc("TRN2", target_bir_lowering=False, debug=False)
    x1_d = nc.dram_tensor("x1", [512, NQ * 2 * Q], f8, kind="ExternalInput").ap()
    x2_d = nc.dram_tensor("x2", [512, NQ * 2 * Q], f8, kind="ExternalInput").ap()
    w1_d = nc.dram_tensor("w1", [512, 2 * 2 * _CH], f8, kind="ExternalInput").ap()
    w2_d = nc.dram_tensor("w2", [512, 2 * 2 * _CH], f8, kind="ExternalInput").ap()
    h_d = nc.dram_tensor("h", [_CH, _S], f32, kind="ExternalOutput").ap()

    # pass list: (x digit, w digit)
    pass_xw = [(0, 0), (1, 0), (0, 1)][:passes]
    nmm = passes * NKP

    with tile.TileContext(nc) as tc:
        with (
            tc.tile_pool(name="wp", bufs=1) as wp,
            tc.tile_pool(name="xp", bufs=xbufs) as xp,
            tc.tile_pool(name="ps", bufs=psbufs, space="PSUM") as ps,
            tc.tile_pool(name="sb", bufs=sbufs) as sb,
            tc.tile_pool(name="hp", bufs=hbufs) as hp,
        ):
            wts = []
            for wi, wd in enumerate((w1_d, w2_d)):
                if wi > 0 and passes < 3:
                    break
                row = []
                for k in range(NKP):
                    t = wp.tile([128, 2, 2 * _CH], f8, tag=f"w{wi}{k}",
                                name=f"w{wi}{k}")
                    nc.sync.dma_start(
                        t[:],
                        wd[k * 128:(k + 1) * 128, :]
                        .rearrange("p (two e) -> p two e", two=2),
                    )
                    row.append(t)
                wts.append(row)

            hprev = [None] * _NP

            def _body_quarter(q):
                sq = slice(q * Q, (q + 1) * Q)
                xts = []
                for xi, xd in enumerate((x1_d, x2_d)):
                    if xi > 0 and passes < 2:
                        break
                    row = []
                    for k in range(NKP):
                        t = xp.tile([128, 2, Q], f8, tag=f"x{xi}{k}",
                                    name=f"x{xi}{k}")
                        src = xd[k * 128:(k + 1) * 128,
                                 q * 2 * Q:(q + 1) * 2 * Q]
                        nc.sync.dma_start(
                            t[:], src.rearrange("p (two j) -> p two j", two=2)
                        )
                        row.append(t)
                    xts.append(row)
                if korder:
                    seq = [(xi, wi, k) for k in range(NKP)
                           for (xi, wi) in pass_xw]
                else:
                    seq = [(xi, wi, k) for (xi, wi) in pass_xw
                           for k in range(NKP)]
                for p in range(_NP):
                    psh = [ps.tile([128, 512], f32, tag=f"ph{h}",
                                   name=f"psh{h}") for h in range(2)]
                    psg = [ps.tile([128, 512], f32, tag=f"pg{h}",
                                   name=f"psg{h}") for h in range(2)]
                    for ec, pst in ((p, psh), (_NP + p, psg)):
                        wcol = slice(ec * 128, (ec + 1) * 128)
                        for half in range(2):
                            xs2 = slice(half * 512, (half + 1) * 512)
                            for n, (xi, wi, k) in enumerate(seq):
                                nc.tensor.matmul(
                                    pst[half][:],
                                    wts[wi][k][:, :, wcol],
                                    xts[xi][k][:, :, xs2],
                                    start=(n == 0),
                                    stop=(n == nmm - 1),
                                    perf_mode=PM,
                                    skip_group_check=True,
                                )
                    sh = sb.tile([128, Q], f32, tag="sh")
                    g = sb.tile([128, Q], f32, tag="g")
                    cc = sb.tile([128, Q], f32, tag="c")
                    z = sb.tile([128, Q], f32, tag="z")
                    v = sb.tile([128, Q], f32, tag="v")
                    for hh in range(2):
                        hs = slice(hh * 512, (hh + 1) * 512)
                        nc.scalar.activation(sh[:, hs], psh[hh][:], AF.Sigmoid)
                        nc.vector.scalar_tensor_tensor(
                            g[:, hs], psh[hh][:], 0.5, sh[:, hs],
                            OP.add, OP.max
                        )
                        nc.scalar.activation(
                            cc[:, hs], psg[hh][:], AF.Sigmoid, scale=-1.0
                        )
                        nc.gpsimd.tensor_scalar(
                            z[:, hs], cc[:, hs], -1.0, 1.0, OP.mult, OP.add
                        )
                        nc.gpsimd.tensor_mul(v[:, hs], z[:, hs], g[:, hs])
                    h = hp.tile([128, Q], f32, tag=f"h{p}")
                    init = 0.0 if q == 0 else hprev[p][:, Q - 1:Q]
                    nc.vector.tensor_tensor_scan(
                        h[:], cc[:], v[:], init, OP.mult, OP.add
                    )
                    hprev[p] = h
                    nc.sync.dma_start(h_d[p * 128:(p + 1) * 128, sq], h[:])

            loop_cm = tc.For_i(0, loop) if loop else contextlib.nullcontext()
            with loop_cm:
                for qq in range(NQ * reps):
                    _body_quarter(qq % NQ)

    nc.compile()
    return nc


def _get_program(reps=1, mm16=False, mm_only=False, fp8=False, **cfg):
    key = (reps, mm16, mm_only, fp8, tuple(sorted(cfg.items())))
    if key not in _programs:
        if fp8:
            _programs[key] = _build_program_fp8(reps, **cfg)
        else:
            _programs[key] = _build_program(reps, mm16, mm_only, **cfg)
    return _programs[key]


def _pack_x_fp8(xd, Q=1024):
    # xd [1024 d, 4096 s] (already fp8-valued, any float dtype)
    # -> [512 (kp p), NQ*2*Q (q two j)]
    import ml_dtypes
    nq = _S // Q
    v = xd.reshape(4, 2, 128, nq, Q)          # kp two p q j
    v = v.transpose(0, 2, 3, 1, 4)            # kp p q two j
    return np.ascontiguousarray(
        v.reshape(512, nq * 2 * Q).astype(ml_dtypes.float8_e4m3))


def _pack_w_fp8(wd):
    # wd [1024 d, 1024 e] -> [512 (kp p), 2048 (two e)]
    import ml_dtypes
    v = wd.reshape(4, 2, 128, 2 * _CH)        # kp two p e
    v = v.transpose(0, 2, 1, 3)               # kp p two e
    return np.ascontiguousarray(
        v.reshape(512, 2 * 2 * _CH).astype(ml_dtypes.float8_e4m3))


def _shard_inputs_fp8(x, W):
    import ml_dtypes
    f8 = ml_dtypes.float8_e4m3
    x = np.ascontiguousarray(x, dtype=np.float32)
    W = np.ascontiguousarray(W, dtype=np.float32)
    in_maps = []
    xdig = []
    for b in range(_B):
        xT = np.ascontiguousarray(x[b].T)                 # [d, s] f32
        x1 = xT.astype(f8).astype(np.float32)
        x2 = (xT - x1).astype(f8).astype(np.float32)
        xdig.append((_pack_x_fp8(x1), _pack_x_fp8(x2)))
    for core in range(_B * 2):
        b, f = divmod(core, 2)
        w_slice = np.concatenate(
            [W[f * _CH:(f + 1) * _CH], W[_D + f * _CH:_D + (f + 1) * _CH]],
            axis=0,
        )  # [e_local, d]
        wT = np.ascontiguousarray(w_slice.T)              # [d, e_local] f32
        w1 = wT.astype(f8).astype(np.float32)
        w2 = (wT - w1).astype(f8).astype(np.float32)
        in_maps.append({
            "x1": xdig[b][0], "x2": xdig[b][1],
            "w1": _pack_w_fp8(w1), "w2": _pack_w_fp8(w2),
        })
    return in_maps


def _shard_inputs(x, W, mm16=False):
    mm_np = np.float16 if mm16 else np.float32
    x = np.ascontiguousarray(x, dtype=np.float32)
    W = np.ascontiguousarray(W, dtype=np.float32)
    in_maps = []
    xT = [np.ascontiguousarray(x[b].T.astype(mm_np)) for b in range(_B)]
    for core in range(_B * 2):
        b, f = divmod(core, 2)
        w_slice = np.concatenate(
            [W[f * _CH : (f + 1) * _CH], W[_D + f * _CH : _D + (f + 1) * _CH]],
            axis=0,
        )  # [1024 (e_local), 1024 (d)]
        wT = np.ascontiguousarray(w_slice.T.astype(mm_np))  # [d, e_local]
        in_maps.append({"x": xT[b], "w": wT})
    return in_maps


def _unshard(results):
    out = np.empty((_B, _S, _D), dtype=np.float32)
    for core in range(_B * 2):
        b, f = divmod(core, 2)
        out[b, :, f * _CH : (f + 1) * _CH] = results[core]["h"].T
    return out


_BUILD_KEYS = ("xbufs", "sbufs", "hbufs", "Q", "psbufs", "zpool", "kin", "xres",
               "xmerge", "psplit", "passes", "korder", "loop")


def run_sharded(x, W, reps=1, mm16=False, mm_only=False, fp8=False, **kwargs):
    """Run the SPMD kernel; returns (output, BassKernelResults)."""
    from concourse.bass_utils import run_bass_kernel_spmd

    cfg = {k: kwargs.pop(k) for k in list(kwargs) if k in _BUILD_KEYS}
    run_kwargs = kwargs
    nc = _get_program(reps, mm16, mm_only, fp8, **cfg)
    in_maps = _shard_inputs_fp8(x, W) if fp8 else _shard_inputs(x, W, mm16)
    last_err = None
    for attempt in range(3):
        try:
            res = run_bass_kernel_spmd(nc, in_maps, list(range(_B * 2)), **run_kwargs)
            return _unshard(res.results), res
        except Exception as e:  # transient device errors (NRT_EXEC_UNIT_...)
            last_err = e
    raise last_err


def kernel(x, W):
    # psplit: per-bank PSUM tiles (measured ~25 us faster than the coarse
    # [128,1024] two-bank tiles — finer producer/consumer sync on PE).
    # Its consumer block also computes z = 1-c on the Pool engine.
    out, _ = run_sharded(x, W, mm16=True, psplit=True)
    return out

